# revision 8
# baseline (speedup 1.0000x reference)
"""Trainium2 Bass kernel for nn_JointAttention (infini-attention, GQA, RoPE, rmsnorm).

Self-contained: hardcodes shapes/sharding. Accepts FULL inputs, returns FULL
(out_x, out_a) like the reference.

Sharding: 8 cores = 2 batches x 4 head-groups. Core c handles batch c//4 and
q-heads PAIRS[c%4] (both in the same GQA group -> one kv head per core).
"""

import sys

sys.path.insert(0, "/opt/trn_rl_repo")

import numpy as np

import concourse.bass as bass
import concourse.tile as tile
import concourse.mybir as mybir
from concourse import bacc
from concourse.bass_utils import run_bass_kernel_spmd

F32 = mybir.dt.float32
F32R = mybir.dt.float32r
BF16 = mybir.dt.bfloat16
AF = mybir.ActivationFunctionType
ALU = mybir.AluOpType

DIM = 512
HEADS = 8
KVH = 2
DH = 64
SEG = 1024
NSEG = 8          # joint n = 8192
NSRC = 4096       # rows per source (a then x)
B = 2
EPS = 1e-12

PAIRS = [(0, 2), (4, 6), (1, 3), (5, 7)]

_STATE = {}


def _build_program():
    nc = bacc.Bacc("TRN2", num_devices=8)

    src = nc.dram_tensor("src", [2, NSRC, DIM], F32R, kind="ExternalInput")
    w_d = nc.dram_tensor("w", [128, 2048], F32R, kind="ExternalInput")
    ct_d = nc.dram_tensor("ct8", [128, 4096], F32, kind="ExternalInput")
    st_d = nc.dram_tensor("st8", [128, 4096], F32, kind="ExternalInput")
    id_d = nc.dram_tensor("ident", [128, 128], F32R, kind="ExternalInput")
    idf_d = nc.dram_tensor("identf", [128, 128], F32, kind="ExternalInput")
    gt_d = nc.dram_tensor("gates", [128, 4], F32, kind="ExternalInput")
    out_d = nc.dram_tensor("out", [2, NSRC, 128], F32, kind="ExternalOutput")

    with tile.TileContext(nc) as tc:
        with (
            tc.tile_pool(name="pc", bufs=1) as pc,        # constants
            tc.tile_pool(name="pd", bufs=1) as pd,        # persistent per-seg data
            tc.tile_pool(name="pw2", bufs=2) as pw2,      # working, double buffered
            tc.tile_pool(name="pw3", bufs=3) as pw3,
            tc.tile_pool(name="pm", bufs=1) as pm,      # working, triple buffered
            tc.tile_pool(name="psA", bufs=2, space="PSUM") as psA,   # [128,512] slots
            tc.tile_pool(name="psB", bufs=3, space="PSUM") as psB,   # [65->128,1024] slots
        ):
            # ---- constants ----
            w_t = pc.tile([128, 2048], F32R)
            nc.sync.dma_start(w_t[:], w_d[:])
            ct_t = pc.tile([128, 4096], F32)
            nc.sync.dma_start(ct_t[:], ct_d[:])
            st_t = pc.tile([128, 4096], F32)
            nc.sync.dma_start(st_t[:], st_d[:])
            id_t = pc.tile([128, 128], F32R)
            nc.sync.dma_start(id_t[:], id_d[:])
            id_f = pc.tile([128, 128], F32)
            nc.sync.dma_start(id_f[:], idf_d[:])
            gt_t = pc.tile([128, 4], F32)
            nc.sync.dma_start(gt_t[:], gt_d[:])
            id_r = id_t

            M_sb = pc.tile([128, 65], F32)
            nc.vector.memset(M_sb[:], 0.0)

            # persistent per-segment tensors
            QT = [pd.tile([128, SEG], F32R, tag=f"QT{i}", name=f"QT{i}") for i in range(NSEG)]
            KT = [pd.tile([128, SEG], F32R, tag=f"KT{i}", name=f"KT{i}") for i in range(NSEG)]
            VA = [pd.tile([128, 8, 65], BF16, tag=f"VA{i}", name=f"VA{i}") for i in range(NSEG)]
            SK = [pd.tile([128, 8, 128], BF16, tag=f"SK{i}", name=f"SK{i}") for i in range(NSEG)]
            for i in range(NSEG):
                nc.vector.memset(VA[i][:, :, 64:65], 1.0)

            # ================= phase 1: proj + rmsnorm + rope =================
            for g in range(64):
                s, nch = g // 32, g % 32
                i, c = g // 8, g % 8

                src_t = pw3.tile([128, DIM], F32R, tag="src")
                nc.sync.dma_start(src_t[:], src[s, nch * 128:(nch + 1) * 128, :])

                xts = []
                for dc in range(4):
                    xt_ps = psA.tile([128, 128], F32, tag="sp")
                    nc.tensor.transpose(
                        xt_ps[:].bitcast(F32R), src_t[:, dc * 128:(dc + 1) * 128], id_r
                    )
                    xt_sb = pw2.tile([128, 128], F32R, tag=f"xts{dc}")
                    nc.vector.tensor_copy(xt_sb[:], xt_ps[:])
                    xts.append(xt_sb)

                proj = psB.tile([128, 256], F32, tag="acc")
                for dc in range(4):
                    o = (s * 4 + dc) * 256
                    nc.tensor.matmul(
                        proj[:], lhsT=xts[dc],
                        rhs=w_t[:, o:o + 256],
                        start=(dc == 0), stop=(dc == 3),
                    )
                proj3 = proj[:, 0:192].rearrange("p (g d) -> p g d", g=3)

                # v (+cast to bf16)
                nc.scalar.activation(VA[i][:, c, 0:64], proj[:, 192:256], AF.Copy)

                # sumsq per group (on raw proj)
                ss = pw2.tile([128, 4], F32, tag="ss")
                sqs = pw2.tile([128, 64], F32, tag="sqs")
                for grp in range(3):
                    nc.scalar.activation(
                        sqs[:], proj3[:, grp], AF.Square, accum_out=ss[:, grp:grp + 1]
                    )
                rinv = pw2.tile([128, 3], F32, tag="rinv")
                nc.scalar.activation(rinv[:], ss[:, 0:3], AF.Sqrt)
                nc.vector.reciprocal(rinv[:], rinv[:])
                nc.vector.tensor_scalar_min(rinv[:], rinv[:], 1e12)

                # rotate-half folded into strided products (sign folded in st8)
                ct_b = ct_t[:, g * 64:(g + 1) * 64][:, None, :].to_broadcast([128, 3, 64])
                st_lo = st_t[:, g * 64:g * 64 + 32][:, None, :].to_broadcast([128, 3, 32])
                st_hi = st_t[:, g * 64 + 32:(g + 1) * 64][:, None, :].to_broadcast([128, 3, 32])
                rot = pw2.tile([128, 3, 64], F32, tag="rot")
                nc.vector.tensor_tensor(rot[:, :, 0:32], proj3[:, :, 32:64], st_lo, ALU.mult)
                nc.vector.tensor_tensor(rot[:, :, 32:64], proj3[:, :, 0:32], st_hi, ALU.mult)
                rope = pw2.tile([128, 3, 64], F32R, tag="rope")
                nc.vector.tensor_tensor(rope[:], proj3[:], ct_b, ALU.mult)
                nc.vector.tensor_add(rope[:], rope[:], rot[:])
                for grp in range(3):
                    nc.vector.tensor_scalar_mul(
                        rope[:, grp], rope[:, grp], rinv[:, grp:grp + 1]
                    )

                # sk = elu(k)+1 = max(k,0) + exp(min(k,0))   (bf16 out)
                mn = pw2.tile([128, 64], F32, tag="mn")
                nc.vector.tensor_scalar_min(mn[:], rope[:, 2], 0.0)
                ex = pw2.tile([128, 64], F32, tag="ex")
                nc.scalar.activation(ex[:], mn[:], AF.Exp)
                nc.vector.scalar_tensor_tensor(
                    SK[i][:, c, 0:64], rope[:, 2], 0.0, ex[:], ALU.max, ALU.add
                )
                nc.gpsimd.tensor_copy(SK[i][:, c, 64:128], SK[i][:, c, 0:64])

                ropef = rope.rearrange("p g d -> p (g d)")
                qtr = psA.tile([128, 128], F32, tag="sp")
                nc.tensor.transpose(qtr[:].bitcast(F32R), ropef[:, 0:128], id_r)
                nc.scalar.activation(QT[i][:, c * 128:(c + 1) * 128], qtr[:], AF.Copy)
                kdup = pw2.tile([128, 128], F32R, tag="kdup")
                nc.gpsimd.tensor_copy(kdup[:, 0:64], rope[:, 2])
                nc.gpsimd.tensor_copy(kdup[:, 64:128], rope[:, 2])
                ktr = psA.tile([128, 128], F32, tag="sp")
                nc.tensor.transpose(ktr[:].bitcast(F32R), kdup[:], id_r)
                nc.vector.tensor_copy(KT[i][:, c * 128:(c + 1) * 128], ktr[:])

            # ================= phase 2: segment recurrence =================
            for i in range(NSEG):
                # sq^T = elu(q^T)+1, bf16
                scr = pw2.tile([128, SEG], F32, tag="sq32")
                nc.vector.tensor_scalar_min(scr[:], QT[i][:], 0.0)
                sqe = pw2.tile([128, SEG], F32, tag="sq32")
                nc.scalar.activation(sqe[:], scr[:], AF.Exp)
                sqb = pw2.tile([128, SEG], BF16, tag="sqb")
                nc.vector.scalar_tensor_tensor(
                    sqb[:], QT[i][:], 0.0, sqe[:], ALU.max, ALU.add
                )
                mb = pw2.tile([128, 65], BF16, tag="maug")
                nc.scalar.activation(mb[:], M_sb[:], AF.Copy)

                msbs, psbs = [], []
                for h in (0, 1):
                    hq = slice(64 * h, 64 * h + 64)
                    mem_ps = psB.tile([65, SEG], F32, tag="acc")
                    for (lo, hi) in ((0, 512), (512, 1024)):
                        nc.tensor.matmul(
                            mem_ps[:, lo:hi], lhsT=mb[hq, :], rhs=sqb[hq, lo:hi],
                            start=True, stop=True,
                        )
                    pv_ps = psB.tile([65, SEG], F32, tag="acc")
                    for c in range(8):
                        c0 = 128 * c
                        E_t = pw3.tile([128, SEG], BF16, tag="E")
                        sblocks = (
                            [(min(c0, 256), 512), (512, 1024)] if c0 < 512
                            else [(min(c0, 768), 1024)]
                        )
                        for (lo, hi) in sblocks:
                            sp = psA.tile([128, 512], F32, tag="sp")
                            nc.tensor.matmul(
                                sp[:, 0:hi - lo],
                                lhsT=KT[i][hq, c0:c0 + 128],
                                rhs=QT[i][hq, lo:hi],
                                start=True, stop=True,
                            )
                            vlo = max(lo, c0)
                            nc.scalar.activation(
                                E_t[:, vlo:hi], sp[:, vlo - lo:hi - lo],
                                AF.Exp, scale=0.125,
                            )
                        # causal mask on diagonal block: keep col>=row
                        nc.gpsimd.affine_select(
                            out=E_t[:, c0:c0 + 128], in_=E_t[:, c0:c0 + 128],
                            pattern=[[1, 128]], compare_op=ALU.is_ge,
                            fill=0.0, base=0, channel_multiplier=-1,
                        )
                        pblocks = [(c0, 512), (512, 1024)] if c < 4 else [(c0, 1024)]
                        for (lo, hi) in pblocks:
                            nc.tensor.matmul(
                                pv_ps[:, lo:hi], lhsT=VA[i][:, c, :],
                                rhs=E_t[:, lo:hi],
                                start=(c == 0),
                                stop=(c == 3 if hi == 512 else c == 7),
                            )
                    mem_sb = pm.tile([65, SEG], F32, tag=f"m{h}")
                    nc.scalar.activation(mem_sb[:], mem_ps[:], AF.Copy)
                    pv_sb = pm.tile([65, SEG], F32, tag=f"p{h}")
                    nc.vector.tensor_copy(pv_sb[:], pv_ps[:])
                    msbs.append(mem_sb)
                    psbs.append(pv_sb)

                # combine + output
                for nblk in range(8):
                    nb = slice(128 * nblk, 128 * nblk + 128)
                    tr = psB.tile([128, 260], F32, tag="acc")
                    for h in (0, 1):
                        nc.tensor.transpose(
                            tr[:, 130 * h:130 * h + 65],
                            msbs[h][:, nb], id_f[0:65, 0:65],
                        )
                        nc.tensor.transpose(
                            tr[:, 130 * h + 65:130 * h + 130],
                            psbs[h][:, nb], id_f[0:65, 0:65],
                        )
                    ob = pw3.tile([128, 128], F32, tag="ob")
                    tr3 = tr.rearrange("p (x y) -> p x y", y=65)
                    for h in (0, 1):
                        rd = pw2.tile([128, 4], F32, tag="rd")
                        nc.vector.tensor_scalar_add(
                            rd[:, 0:2], tr3[:, 2 * h:2 * h + 2, 64], EPS
                        )
                        nc.vector.reciprocal(rd[:, 2:4], rd[:, 0:2])
                        nc.vector.tensor_tensor(
                            rd[:, 2:4], rd[:, 2:4],
                            gt_t.rearrange("p (x y) -> p x y", y=2)[:, :, h],
                            ALU.mult,
                        )
                        tmp = pw2.tile([128, 64], F32, tag="tmp")
                        nc.vector.tensor_scalar_mul(
                            tmp[:], tr[:, 130 * h:130 * h + 64], rd[:, 2:3]
                        )
                        nc.vector.scalar_tensor_tensor(
                            ob[:, 64 * h:64 * h + 64],
                            tr[:, 130 * h + 65:130 * h + 129],
                            rd[:, 3:4], tmp[:], ALU.mult, ALU.add,
                        )
                    s_out, loc = i // 4, SEG * (i % 4) + 128 * nblk
                    nc.sync.dma_start(out_d[s_out, loc:loc + 128, :], ob[:])

                # M update
                mupd = psB.tile([128, 65], F32, tag="acc")
                for c in range(8):
                    nc.tensor.matmul(
                        mupd[:], lhsT=SK[i][:, c, :], rhs=VA[i][:, c, :],
                        start=(c == 0), stop=(c == 7),
                    )
                nc.vector.tensor_add(M_sb[:], M_sb[:], mupd[:])

    nc.compile()
    return nc


def _host_inputs(inputs):
    """Build per-core in_maps from the full problem inputs."""
    x = np.asarray(inputs["x"], np.float32)
    a = np.asarray(inputs["a"], np.float32)
    beta = np.asarray(inputs["beta"], np.float32)

    # rope tables, gamma(=1)*sqrt(dh) folded, sign of sin folded for rotate-half
    pos = np.arange(2 * NSRC, dtype=np.float64)
    half = DH // 2
    inv_freq = 1.0 / (10000.0 ** (np.arange(half, dtype=np.float64) / half))
    fr = pos[:, None] * inv_freq[None, :]
    cos = np.concatenate([np.cos(fr)] * 2, 1)
    sin = np.concatenate([np.sin(fr)] * 2, 1)
    sgn = np.ones((1, DH)); sgn[0, :half] = -1.0
    ct8 = (8.0 * cos).astype(np.float32)
    st8 = (8.0 * sin * sgn).astype(np.float32)
    ct8 = ct8.reshape(64, 128, 64).transpose(1, 0, 2).reshape(128, 4096)
    st8 = st8.reshape(64, 128, 64).transpose(1, 0, 2).reshape(128, 4096)

    ident = np.eye(128, dtype=np.float32)
    g = 1.0 / (1.0 + np.exp(-beta.astype(np.float64)))

    in_maps = []
    for core in range(8):
        b, j = core // 4, core % 4
        h0, h1 = PAIRS[j]
        kv = h0 % KVH
        src = np.stack([a[b], x[b]])  # [2, 4096, 512]
        ws = []
        for wq, wk, wv in ((inputs["Wq_a"], inputs["Wk_a"], inputs["Wv_a"]),
                           (inputs["Wq_x"], inputs["Wk_x"], inputs["Wv_x"])):
            wq = np.asarray(wq, np.float32); wk = np.asarray(wk, np.float32)
            wv = np.asarray(wv, np.float32)
            ws.append(np.concatenate(
                [wq[:, h0 * DH:(h0 + 1) * DH], wq[:, h1 * DH:(h1 + 1) * DH],
                 wk[:, kv * DH:(kv + 1) * DH], wv[:, kv * DH:(kv + 1) * DH]], 1))
        w_all = np.stack(ws)  # [2, 512, 256]
        w_host = np.ascontiguousarray(
            w_all.reshape(2, 4, 128, 256).transpose(2, 0, 1, 3).reshape(128, 2048))
        gates = np.tile(np.array(
            [g[h0], g[h1], 1 - g[h0], 1 - g[h1]], np.float32), (128, 1))
        in_maps.append({
            "src": np.ascontiguousarray(src),
            "w": w_host,
            "ct8": ct8, "st8": st8, "ident": ident, "identf": ident,
            "gates": np.ascontiguousarray(gates),
        })
    return in_maps


def _check_fastpath(inputs):
    for k in ("gq_x", "gk_x", "gq_a", "gk_a"):
        if not np.allclose(np.asarray(inputs[k]), 1.0):
            raise NotImplementedError("kernel assumes unit rmsnorm gamma")


def kernel(**inputs):
    _check_fastpath(inputs)
    if "nc" not in _STATE:
        _STATE["nc"] = _build_program()
    nc = _STATE["nc"]
    in_maps = _host_inputs(inputs)
    res = run_bass_kernel_spmd(nc, in_maps, core_ids=list(range(8)))

    out_x = np.zeros((B, NSRC, DIM), np.float32)
    out_a = np.zeros((B, NSRC, DIM), np.float32)
    for core in range(8):
        b, j = core // 4, core % 4
        h0, h1 = PAIRS[j]
        o = res.results[core]["out"]  # [2, 4096, 128]
        out_a[b, :, h0 * DH:(h0 + 1) * DH] = o[0, :, 0:64]
        out_a[b, :, h1 * DH:(h1 + 1) * DH] = o[0, :, 64:128]
        out_x[b, :, h0 * DH:(h0 + 1) * DH] = o[1, :, 0:64]
        out_x[b, :, h1 * DH:(h1 + 1) * DH] = o[1, :, 64:128]
    return out_x, out_a
